# revision 40
# baseline (speedup 1.0000x reference)
"""Trainium2 Bass kernel for masked-softmax attention scoring.

Reference computation (B=128, T=512, K=1024, Q=1024):
    mids  = einsum("kq,bq->bk", W, query)
    s     = tanh(einsum("btk,bk->bt", key, mids) + bias)
    attn  = softmax-like: exp(s - max) * mask / sum(exp(s - max) * mask)

The max-subtraction cancels exactly in the ratio (tanh is bounded), so the
device computes  attn = exp(tanh(.)) * mask / sum_t(exp(tanh(.)) * mask).

Sharding: data-parallel over B across 8 NeuronCores (16 batches/core).
Per-core layout: partition p = (b, j) with b in [0,16), j in [0,8);
free column c in [0,64); timestep t = j*64 + c.

Pipeline (single sync HWDGE ring, ~410 GB/s sustained):
 - ring order [qt, W^T x8, key col 0..63] guarantees W fully lands before
   any key bytes compete for bandwidth (the SDMA packet round-robin is not
   fair across rings, so a second ring is used only for the tiny
   mask/bias/grp loads).
 - W and query are fp16 (host-cast): halves W bytes and fp16 matmuls are
   single-pass (fp32 is a LOW+HIGH double pass).  PE warm-up matmuls on
   garbage data raise the PE pstate before the real mids chain.
 - mids matmul: stationary = 8x-replicated query columns; 16 accumulating
   matmuls paced by W chunk arrivals; PSUM -> SBUF copy on DVE.
 - each key column is consumed by ONE fused DVE multiply-reduce
   (native scalar_tensor_tensor, ~1.22 us/col = the stream rate; every
   DVE op with an accumulator runs 1x, and GpSimd/ACT cannot help:
   GpSimd tensor ops wreck DVE throughput via SBUF contention).
 - epilogue: full-width tanh/exp (ACT), one fused mask-mul+rowsum (DVE),
   group-sum via block-diagonal 0/1 fp16 matmul, reciprocal, scale, store.
"""

import sys

if "/opt/trn_rl_repo" not in sys.path:
    sys.path.insert(0, "/opt/trn_rl_repo")

from contextlib import ExitStack

import numpy as np

# ---- problem constants (hardcoded per spec) ----
B, T, K, Q = 128, 512, 1024, 1024
NCORES = 8
BS = B // NCORES          # 16 batches per core
P = 128                   # SBUF partitions
J = P // BS               # 8 t-blocks per batch on partitions
CF = T // J               # 64 timesteps per (partition, free col)
QC = Q // P               # 8 contraction chunks for the mids matmul
KEY_BUFS = 24             # key column pool depth (512 KB per slot)
N_WARM = 6                # PE pstate warm-up matmuls

_STATE: dict = {}


def _build_nc():
    import concourse.tile as tile
    from concourse import bacc, mybir

    f32 = mybir.dt.float32
    f16 = mybir.dt.float16
    mul = mybir.AluOpType.mult
    byp = mybir.AluOpType.bypass
    nc = bacc.Bacc()

    qt_e = nc.declare_dram_parameter("qt", [P, QC, BS], f16, isOutput=False)
    wt_e = nc.declare_dram_parameter("wt", [P, QC, K], f16, isOutput=False)
    grp_e = nc.declare_dram_parameter("grp", [P, P], f16, isOutput=False)
    key_e = nc.declare_dram_parameter("key", [BS, T, K], f32, isOutput=False)
    mb_e = nc.declare_dram_parameter("maskbias", [P, CF + 1], f32, isOutput=False)
    out_e = nc.declare_dram_parameter("out", [P, CF], f32, isOutput=True)

    with tile.TileContext(nc) as tc, ExitStack() as ctx:
        const = ctx.enter_context(tc.tile_pool(name="const", bufs=1))
        kpool = ctx.enter_context(tc.tile_pool(name="key", bufs=KEY_BUFS))
        vpool = ctx.enter_context(tc.tile_pool(name="vprod", bufs=2))
        psum = ctx.enter_context(tc.tile_pool(name="psum", bufs=1, space="PSUM"))

        # ---- PE pstate warm-up: garbage matmuls while the ring fills ----
        warm_sb = const.tile([P, 512], f16)
        nc.gpsimd.memset(warm_sb[:], 1.0)
        warm_ps = psum.tile([P, 512], f32)
        for _ in range(N_WARM):
            nc.tensor.matmul(
                warm_ps[:],
                lhsT=warm_sb[:, 0:P],
                rhs=warm_sb[:],
                start=True,
                stop=True,
            )

        # ---- prologue: everything big on the sync ring, tiny loads on the
        # scalar ring (cross-ring packet arbitration is unfair, so W must
        # not share a ring with key traffic).
        qt_sb = const.tile([P, QC, BS], f16)
        nc.sync.dma_start(out=qt_sb[:], in_=qt_e[:])
        mb_sb = const.tile([P, CF + 1], f32)
        nc.scalar.dma_start(out=mb_sb[:], in_=mb_e[:])
        grp_sb = const.tile([P, P], f16)
        nc.scalar.dma_start(out=grp_sb[:], in_=grp_e[:])
        maskr_sb = mb_sb[:, 0:CF]
        bias_sb = mb_sb[:, CF : CF + 1]
        wt_sb = const.tile([P, QC, K], f16)
        for qc in range(QC):
            nc.sync.dma_start(out=wt_sb[:, qc, :], in_=wt_e[:, qc, :])

        # ---- mids in broadcast layout: [P, K], row p = mids[b(p), :] ----
        # Replicate each query column 8x on-chip (stride-0 DVE read) so the
        # stationary operand has the (b, j) partition order in one free dim.
        qtrep_sb = const.tile([P, QC, BS, J], f16)
        nc.vector.tensor_copy(
            qtrep_sb[:], qt_sb[:].unsqueeze(-1).broadcast_to((P, QC, BS, J))
        )
        mids_ps = psum.tile([P, K], f32)
        for qi, qc in enumerate(range(QC)):
            lhsT = qtrep_sb[:, qc, :, :]
            for h in range(2):
                nc.tensor.matmul(
                    mids_ps[:, h * 512 : (h + 1) * 512],
                    lhsT=lhsT,
                    rhs=wt_sb[:, qc, h * 512 : (h + 1) * 512],
                    start=(qi == 0),
                    stop=(qi == QC - 1),
                )
        mids_bc = const.tile([P, K], f32)
        nc.vector.tensor_copy(mids_bc[:], mids_ps[:])

        # ---- scores[p, c] = key[b, j*64+c, :] . mids[b, :] ----
        # 64 per-column 512 KB DMAs on the sync ring behind W; one fused
        # DVE multiply-reduce per column at the stream rate.
        scores_sb = const.tile([P, CF], f32)
        tanh_sb = const.tile([P, CF], f32)
        em_sb = const.tile([P, CF], f32)
        key_r = key_e[:].rearrange("b (j c) k -> (b j) c k", j=J)
        for c in range(CF):
            kt = kpool.tile([P, K], f32, tag="k")
            nc.sync.dma_start(out=kt[:], in_=key_r[:, c, :])
            prod = vpool.tile([P, K], f32, tag="v")
            nc.vector.scalar_tensor_tensor(
                out=prod[:],
                in0=kt[:],
                scalar=0.0,
                in1=mids_bc[:],
                op0=byp,
                op1=mul,
                accum_out=scores_sb[:, c : c + 1],
            )

        # ---- epilogue: tanh, exp, mask, normalize ----
        nc.scalar.activation(
            out=tanh_sb[:],
            in_=scores_sb[:],
            func=mybir.ActivationFunctionType.Tanh,
            bias=bias_sb,
            scale=1.0,
        )
        nc.scalar.activation(
            out=em_sb[:],
            in_=tanh_sb[:],
            func=mybir.ActivationFunctionType.Exp,
        )
        emm_sb = const.tile([P, CF], f32)
        rowsum = const.tile([P, 1], f32)
        nc.vector.scalar_tensor_tensor(
            out=emm_sb[:],
            in0=em_sb[:],
            scalar=0.0,
            in1=maskr_sb,
            op0=byp,
            op1=mul,
            accum_out=rowsum[:],
        )
        rowsum16 = const.tile([P, 1], f16)
        nc.vector.tensor_copy(rowsum16[:], rowsum[:])
        den_ps = psum.tile([P, 1], f32)
        nc.tensor.matmul(
            den_ps[:], lhsT=grp_sb[:], rhs=rowsum16[:], start=True, stop=True
        )
        rinv = const.tile([P, 1], f32)
        nc.vector.reciprocal(out=rinv[:], in_=den_ps[:])
        attn_sb = const.tile([P, CF], f32)
        nc.vector.tensor_scalar_mul(attn_sb[:], emm_sb[:], rinv[:])
        nc.sync.dma_start(out=out_e[:], in_=attn_sb[:])

    nc.compile()
    return nc


def _get_nc():
    if "nc" not in _STATE:
        _STATE["nc"] = _build_nc()
    return _STATE["nc"]


def _grp():
    if "GRP" not in _STATE:
        # GRP[p, m] = 1 iff p // J == m // J  (block-diagonal group-sum)
        pj = np.arange(P) // J
        _STATE["GRP"] = np.ascontiguousarray(
            (pj[:, None] == pj[None, :]).astype(np.float16)
        )
    return _STATE["GRP"]


def _make_in_maps(query, key, mask, W, bias):
    query = np.asarray(query, dtype=np.float32)
    key = np.asarray(key, dtype=np.float32)
    mask = np.asarray(mask, dtype=np.float32)
    W = np.asarray(W, dtype=np.float32)
    bias = np.asarray(bias, dtype=np.float32).reshape(-1)

    # wt[p, qc, k] = W.T[qc*128 + p, k]
    WT = np.ascontiguousarray(
        np.ascontiguousarray(W.T).reshape(QC, P, K).transpose(1, 0, 2)
    ).astype(np.float16)
    GRP = _grp()

    in_maps = []
    for i in range(NCORES):
        sh = slice(i * BS, (i + 1) * BS)
        maskbias = np.concatenate(
            [
                np.ascontiguousarray(mask[sh]).reshape(P, CF),
                np.broadcast_to(bias[:1][None, :], (P, 1)),
            ],
            axis=1,
        ).astype(np.float32)
        in_maps.append(
            {
                # pre-laid [P, QC, BS]: qt[p, qc, b] = query[sh].T[qc*128+p, b]
                "qt": np.ascontiguousarray(
                    query[sh].T.reshape(QC, P, BS).transpose(1, 0, 2)
                ).astype(np.float16),
                "wt": WT,
                "grp": GRP,
                "key": np.ascontiguousarray(key[sh]),
                "maskbias": np.ascontiguousarray(maskbias),
            }
        )
    return in_maps


def _run(in_maps, **kwargs):
    from concourse.bass_utils import run_bass_kernel_spmd

    return run_bass_kernel_spmd(
        _get_nc(), in_maps, core_ids=list(range(NCORES)), **kwargs
    )


def _gather(results):
    return np.concatenate(
        [np.asarray(r["out"]).reshape(BS, T) for r in results], axis=0
    )


def kernel(query, key, mask, W, bias):
    in_maps = _make_in_maps(query, key, mask, W, bias)
    res = _run(in_maps)
    return _gather(res.results)


# revision 41
# speedup vs baseline: 1.0295x; 1.0295x over previous
"""Trainium2 Bass kernel for masked-softmax attention scoring.

Reference computation (B=128, T=512, K=1024, Q=1024):
    mids  = einsum("kq,bq->bk", W, query)
    s     = tanh(einsum("btk,bk->bt", key, mids) + bias)
    attn  = softmax-like: exp(s - max) * mask / sum(exp(s - max) * mask)

The max-subtraction cancels exactly in the ratio (tanh is bounded), so the
device computes  attn = exp(tanh(.)) * mask / sum_t(exp(tanh(.)) * mask).

Sharding: data-parallel over B across 8 NeuronCores (16 batches/core).
Per-core layout: partition p = (b, j) with b in [0,16), j in [0,8);
free column c in [0,64); timestep t = j*64 + c.

Pipeline (single sync HWDGE ring, ~410 GB/s sustained):
 - ring order [qt, W^T x8, key col 0..63] guarantees W fully lands before
   any key bytes compete for bandwidth (the SDMA packet round-robin is not
   fair across rings, so a second ring is used only for the tiny
   mask/bias/grp loads).
 - W and query are fp16 (host-cast): halves W bytes and fp16 matmuls are
   single-pass (fp32 is a LOW+HIGH double pass).  PE warm-up matmuls on
   garbage data raise the PE pstate before the real mids chain.
 - mids matmul: stationary = 8x-replicated query columns; 16 accumulating
   matmuls paced by W chunk arrivals; PSUM -> SBUF copy on DVE.
 - each key column is consumed by ONE fused DVE multiply-reduce
   (native scalar_tensor_tensor, ~1.22 us/col = the stream rate; every
   DVE op with an accumulator runs 1x, and GpSimd/ACT cannot help:
   GpSimd tensor ops wreck DVE throughput via SBUF contention).
 - epilogue: full-width tanh/exp (ACT), one fused mask-mul+rowsum (DVE),
   group-sum via block-diagonal 0/1 fp16 matmul, reciprocal, scale, store.
"""

import sys

if "/opt/trn_rl_repo" not in sys.path:
    sys.path.insert(0, "/opt/trn_rl_repo")

from contextlib import ExitStack

import numpy as np

# ---- problem constants (hardcoded per spec) ----
B, T, K, Q = 128, 512, 1024, 1024
NCORES = 8
BS = B // NCORES          # 16 batches per core
P = 128                   # SBUF partitions
J = P // BS               # 8 t-blocks per batch on partitions
CF = T // J               # 64 timesteps per (partition, free col)
QC = Q // P               # 8 contraction chunks for the mids matmul
KEY_BUFS = 28             # key column pool depth (512 KB per slot)
N_WARM = 6                # PE pstate warm-up matmuls

_STATE: dict = {}


def _build_nc():
    import concourse.tile as tile
    from concourse import bacc, mybir

    f32 = mybir.dt.float32
    f16 = mybir.dt.float16
    mul = mybir.AluOpType.mult
    byp = mybir.AluOpType.bypass
    nc = bacc.Bacc()

    qt_e = nc.declare_dram_parameter("qt", [P, QC, BS], f16, isOutput=False)
    wt_e = nc.declare_dram_parameter("wt", [P, QC, K], f16, isOutput=False)
    grp_e = nc.declare_dram_parameter("grp", [P, P], f16, isOutput=False)
    key_e = nc.declare_dram_parameter("key", [BS, T, K], f32, isOutput=False)
    mb_e = nc.declare_dram_parameter("maskbias", [P, CF + 1], f32, isOutput=False)
    out_e = nc.declare_dram_parameter("out", [P, CF], f32, isOutput=True)

    with tile.TileContext(nc) as tc, ExitStack() as ctx:
        const = ctx.enter_context(tc.tile_pool(name="const", bufs=1))
        kpool = ctx.enter_context(tc.tile_pool(name="key", bufs=KEY_BUFS))
        vpool = ctx.enter_context(tc.tile_pool(name="vprod", bufs=2))
        psum = ctx.enter_context(tc.tile_pool(name="psum", bufs=1, space="PSUM"))

        # ---- PE pstate warm-up: garbage matmuls while the ring fills ----
        warm_sb = const.tile([P, 512], f16)
        nc.gpsimd.memset(warm_sb[:], 1.0)
        warm_ps = psum.tile([P, 512], f32)
        for _ in range(N_WARM):
            nc.tensor.matmul(
                warm_ps[:],
                lhsT=warm_sb[:, 0:P],
                rhs=warm_sb[:],
                start=True,
                stop=True,
            )

        # ---- prologue: everything big on the sync ring, tiny loads on the
        # scalar ring (cross-ring packet arbitration is unfair, so W must
        # not share a ring with key traffic).
        qt_sb = const.tile([P, QC, BS], f16)
        nc.sync.dma_start(out=qt_sb[:], in_=qt_e[:])
        mb_sb = const.tile([P, CF + 1], f32)
        nc.scalar.dma_start(out=mb_sb[:], in_=mb_e[:])
        grp_sb = const.tile([P, P], f16)
        nc.scalar.dma_start(out=grp_sb[:], in_=grp_e[:])
        maskr_sb = mb_sb[:, 0:CF]
        bias_sb = mb_sb[:, CF : CF + 1]
        wt_sb = const.tile([P, QC, K], f16)
        for qc in range(QC):
            nc.sync.dma_start(out=wt_sb[:, qc, :], in_=wt_e[:, qc, :])

        # ---- mids in broadcast layout: [P, K], row p = mids[b(p), :] ----
        # Replicate each query column 8x on-chip (stride-0 DVE read) so the
        # stationary operand has the (b, j) partition order in one free dim.
        qtrep_sb = const.tile([P, QC, BS, J], f16)
        nc.vector.tensor_copy(
            qtrep_sb[:], qt_sb[:].unsqueeze(-1).broadcast_to((P, QC, BS, J))
        )
        mids_ps = psum.tile([P, K], f32)
        for qi, qc in enumerate(range(QC)):
            lhsT = qtrep_sb[:, qc, :, :]
            for h in range(2):
                nc.tensor.matmul(
                    mids_ps[:, h * 512 : (h + 1) * 512],
                    lhsT=lhsT,
                    rhs=wt_sb[:, qc, h * 512 : (h + 1) * 512],
                    start=(qi == 0),
                    stop=(qi == QC - 1),
                )
        mids_bc = const.tile([P, K], f32)
        nc.vector.tensor_copy(mids_bc[:], mids_ps[:])

        # ---- scores[p, c] = key[b, j*64+c, :] . mids[b, :] ----
        # 64 per-column 512 KB DMAs on the sync ring behind W; one fused
        # DVE multiply-reduce per column at the stream rate.
        scores_sb = const.tile([P, CF], f32)
        tanh_sb = const.tile([P, CF], f32)
        em_sb = const.tile([P, CF], f32)
        key_r = key_e[:].rearrange("b (j c) k -> (b j) c k", j=J)
        H = CF // 2
        for c in range(CF):
            kt = kpool.tile([P, K], f32, tag="k")
            nc.sync.dma_start(out=kt[:], in_=key_r[:, c, :])
            prod = vpool.tile([P, K], f16, tag="v")
            nc.vector.scalar_tensor_tensor(
                out=prod[:],
                in0=kt[:],
                scalar=0.0,
                in1=mids_bc[:],
                op0=byp,
                op1=mul,
                accum_out=scores_sb[:, c : c + 1],
            )
            if c == H - 1 or c == CF - 1:
                # tanh+exp per half on the (idle) ACT engine: the first
                # half overlaps the stream, only the second is tail work
                g0, g1 = (0, H) if c == H - 1 else (H, CF)
                nc.scalar.activation(
                    out=tanh_sb[:, g0:g1],
                    in_=scores_sb[:, g0:g1],
                    func=mybir.ActivationFunctionType.Tanh,
                    bias=bias_sb,
                    scale=1.0,
                )
                nc.scalar.activation(
                    out=em_sb[:, g0:g1],
                    in_=tanh_sb[:, g0:g1],
                    func=mybir.ActivationFunctionType.Exp,
                )

        # ---- epilogue: mask, normalize ----
        emm_sb = const.tile([P, CF], f32)
        rowsum = const.tile([P, 1], f32)
        nc.vector.scalar_tensor_tensor(
            out=emm_sb[:],
            in0=em_sb[:],
            scalar=0.0,
            in1=maskr_sb,
            op0=byp,
            op1=mul,
            accum_out=rowsum[:],
        )
        rowsum16 = const.tile([P, 1], f16)
        nc.vector.tensor_copy(rowsum16[:], rowsum[:])
        den_ps = psum.tile([P, 1], f32)
        nc.tensor.matmul(
            den_ps[:], lhsT=grp_sb[:], rhs=rowsum16[:], start=True, stop=True
        )
        rinv = const.tile([P, 1], f32)
        nc.vector.reciprocal(out=rinv[:], in_=den_ps[:])
        attn_sb = const.tile([P, CF], f32)
        nc.vector.tensor_scalar_mul(attn_sb[:], emm_sb[:], rinv[:])
        nc.sync.dma_start(out=out_e[:], in_=attn_sb[:])

    nc.compile()
    return nc


def _get_nc():
    if "nc" not in _STATE:
        _STATE["nc"] = _build_nc()
    return _STATE["nc"]


def _grp():
    if "GRP" not in _STATE:
        # GRP[p, m] = 1 iff p // J == m // J  (block-diagonal group-sum)
        pj = np.arange(P) // J
        _STATE["GRP"] = np.ascontiguousarray(
            (pj[:, None] == pj[None, :]).astype(np.float16)
        )
    return _STATE["GRP"]


def _make_in_maps(query, key, mask, W, bias):
    query = np.asarray(query, dtype=np.float32)
    key = np.asarray(key, dtype=np.float32)
    mask = np.asarray(mask, dtype=np.float32)
    W = np.asarray(W, dtype=np.float32)
    bias = np.asarray(bias, dtype=np.float32).reshape(-1)

    # wt[p, qc, k] = W.T[qc*128 + p, k]
    WT = np.ascontiguousarray(
        np.ascontiguousarray(W.T).reshape(QC, P, K).transpose(1, 0, 2)
    ).astype(np.float16)
    GRP = _grp()

    in_maps = []
    for i in range(NCORES):
        sh = slice(i * BS, (i + 1) * BS)
        maskbias = np.concatenate(
            [
                np.ascontiguousarray(mask[sh]).reshape(P, CF),
                np.broadcast_to(bias[:1][None, :], (P, 1)),
            ],
            axis=1,
        ).astype(np.float32)
        in_maps.append(
            {
                # pre-laid [P, QC, BS]: qt[p, qc, b] = query[sh].T[qc*128+p, b]
                "qt": np.ascontiguousarray(
                    query[sh].T.reshape(QC, P, BS).transpose(1, 0, 2)
                ).astype(np.float16),
                "wt": WT,
                "grp": GRP,
                "key": np.ascontiguousarray(key[sh]),
                "maskbias": np.ascontiguousarray(maskbias),
            }
        )
    return in_maps


def _run(in_maps, **kwargs):
    from concourse.bass_utils import run_bass_kernel_spmd

    return run_bass_kernel_spmd(
        _get_nc(), in_maps, core_ids=list(range(NCORES)), **kwargs
    )


def _gather(results):
    return np.concatenate(
        [np.asarray(r["out"]).reshape(BS, T) for r in results], axis=0
    )


def kernel(query, key, mask, W, bias):
    in_maps = _make_in_maps(query, key, mask, W, bias)
    res = _run(in_maps)
    return _gather(res.results)


# revision 42
# speedup vs baseline: 1.1748x; 1.1412x over previous
"""Trainium2 Bass kernel for masked-softmax attention scoring.

Reference computation (B=128, T=512, K=1024, Q=1024):
    mids  = einsum("kq,bq->bk", W, query)
    s     = tanh(einsum("btk,bk->bt", key, mids) + bias)
    attn  = softmax-like: exp(s - max) * mask / sum(exp(s - max) * mask)

The max-subtraction cancels exactly in the ratio (tanh is bounded), so the
device computes  attn = exp(tanh(.)) * mask / sum_t(exp(tanh(.)) * mask).

Sharding: data-parallel over B across 8 NeuronCores (16 batches/core).
Per-core layout: partition p = (b, j) with b in [0,16), j in [0,8);
free column c in [0,64); timestep t = j*64 + c.

Pipeline (single sync HWDGE ring, ~410 GB/s sustained):
 - ring order [qt, W^T x8, key col 0..63] guarantees W fully lands before
   any key bytes compete for bandwidth (the SDMA packet round-robin is not
   fair across rings, so a second ring is used only for the tiny
   mask/bias/grp loads).
 - W and query are fp16 (host-cast): halves W bytes and fp16 matmuls are
   single-pass (fp32 is a LOW+HIGH double pass).  PE warm-up matmuls on
   garbage data raise the PE pstate before the real mids chain.
 - mids matmul: stationary = 8x-replicated query columns; 16 accumulating
   matmuls paced by W chunk arrivals; PSUM -> SBUF copy on DVE.
 - each key column is consumed by ONE fused DVE multiply-reduce
   (native scalar_tensor_tensor, ~1.22 us/col = the stream rate; every
   DVE op with an accumulator runs 1x, and GpSimd/ACT cannot help:
   GpSimd tensor ops wreck DVE throughput via SBUF contention).
 - epilogue: tanh/exp per column-half on ACT (first half overlaps the
   stream), one fused mask-mul+rowsum (DVE), group-sum via block-diagonal
   0/1 fp16 matmul, reciprocal, scale, store.
"""

import sys

if "/opt/trn_rl_repo" not in sys.path:
    sys.path.insert(0, "/opt/trn_rl_repo")

from contextlib import ExitStack

import numpy as np

# ---- problem constants (hardcoded per spec) ----
B, T, K, Q = 128, 512, 1024, 1024
NCORES = 8
BS = B // NCORES          # 16 batches per core
P = 128                   # SBUF partitions
J = P // BS               # 8 t-blocks per batch on partitions
CF = T // J               # 64 timesteps per (partition, free col)
QC = Q // P               # 8 contraction chunks for the mids matmul
KEY_BUFS = 28             # key column pool depth (512 KB per slot)
N_WARM = 6                # PE pstate warm-up matmuls

_STATE: dict = {}


def _build_nc():
    import concourse.tile as tile
    from concourse import bacc, mybir

    f32 = mybir.dt.float32
    f16 = mybir.dt.float16
    mul = mybir.AluOpType.mult
    byp = mybir.AluOpType.bypass
    nc = bacc.Bacc()

    qt_e = nc.declare_dram_parameter("qt", [P, QC, BS], f16, isOutput=False)
    wt_e = nc.declare_dram_parameter("wt", [P, QC, K], f16, isOutput=False)
    grp_e = nc.declare_dram_parameter("grp", [P, P], f16, isOutput=False)
    key_e = nc.declare_dram_parameter("key", [BS, T, K], f32, isOutput=False)
    mb_e = nc.declare_dram_parameter("maskbias", [P, CF + 1], f32, isOutput=False)
    out_e = nc.declare_dram_parameter("out", [P, CF], f32, isOutput=True)

    with tile.TileContext(nc) as tc, ExitStack() as ctx:
        const = ctx.enter_context(tc.tile_pool(name="const", bufs=1))
        kpool = ctx.enter_context(tc.tile_pool(name="key", bufs=KEY_BUFS))
        vpool = ctx.enter_context(tc.tile_pool(name="vprod", bufs=2))
        psum = ctx.enter_context(tc.tile_pool(name="psum", bufs=1, space="PSUM"))

        # ---- PE pstate warm-up: garbage matmuls while the ring fills ----
        warm_sb = const.tile([P, 512], f16)
        nc.gpsimd.memset(warm_sb[:], 1.0)
        warm_ps = psum.tile([P, 512], f32)
        for _ in range(N_WARM):
            nc.tensor.matmul(
                warm_ps[:],
                lhsT=warm_sb[:, 0:P],
                rhs=warm_sb[:],
                start=True,
                stop=True,
            )

        # ---- prologue: everything big on the sync ring, tiny loads on the
        # scalar ring (cross-ring packet arbitration is unfair, so W must
        # not share a ring with key traffic).
        qt_sb = const.tile([P, QC, BS], f16)
        nc.sync.dma_start(out=qt_sb[:], in_=qt_e[:])
        mb_sb = const.tile([P, CF + 1], f32)
        nc.scalar.dma_start(out=mb_sb[:], in_=mb_e[:])
        grp_sb = const.tile([P, P], f16)
        nc.scalar.dma_start(out=grp_sb[:], in_=grp_e[:])
        maskr_sb = mb_sb[:, 0:CF]
        bias_sb = mb_sb[:, CF : CF + 1]
        wt_sb = const.tile([P, QC, K], f16)
        for qc in range(QC):
            nc.sync.dma_start(out=wt_sb[:, qc, :], in_=wt_e[:, qc, :])

        # ---- mids in broadcast layout: [P, K], row p = mids[b(p), :] ----
        # Replicate each query column 8x on-chip (stride-0 DVE read) so the
        # stationary operand has the (b, j) partition order in one free dim.
        qtrep_sb = const.tile([P, QC, BS, J], f16)
        nc.vector.tensor_copy(
            qtrep_sb[:], qt_sb[:].unsqueeze(-1).broadcast_to((P, QC, BS, J))
        )
        mids_ps = psum.tile([P, K], f32)
        for qi, qc in enumerate(range(QC)):
            lhsT = qtrep_sb[:, qc, :, :]
            for h in range(2):
                nc.tensor.matmul(
                    mids_ps[:, h * 512 : (h + 1) * 512],
                    lhsT=lhsT,
                    rhs=wt_sb[:, qc, h * 512 : (h + 1) * 512],
                    start=(qi == 0),
                    stop=(qi == QC - 1),
                )
        mids_bc = const.tile([P, K], f32)
        nc.vector.tensor_copy(mids_bc[:], mids_ps[:])

        # ---- scores[p, c] = key[b, j*64+c, :] . mids[b, :] ----
        # 64 per-column 512 KB DMAs on the sync ring behind W; one fused
        # DVE multiply-reduce per column at the stream rate.
        scores_sb = const.tile([P, CF], f32)
        tanh_sb = const.tile([P, CF], f32)
        em_sb = const.tile([P, CF], f32)
        key_r = key_e[:].rearrange("b (j c) k -> (b j) c k", j=J)
        H = CF // 2
        for c in range(CF):
            kt = kpool.tile([P, K], f32, tag="k")
            nc.sync.dma_start(out=kt[:], in_=key_r[:, c, :])
            prod = vpool.tile([P, K], f16, tag="v")
            nc.vector.scalar_tensor_tensor(
                out=prod[:],
                in0=kt[:],
                scalar=0.0,
                in1=mids_bc[:],
                op0=byp,
                op1=mul,
                accum_out=scores_sb[:, c : c + 1],
            )
            if c == H - 1 or c == CF - 1:
                # tanh+exp per half on the (idle) ACT engine: the first
                # half overlaps the stream, only the second is tail work
                g0, g1 = (0, H) if c == H - 1 else (H, CF)
                nc.scalar.activation(
                    out=tanh_sb[:, g0:g1],
                    in_=scores_sb[:, g0:g1],
                    func=mybir.ActivationFunctionType.Tanh,
                    bias=bias_sb,
                    scale=1.0,
                )
                nc.scalar.activation(
                    out=em_sb[:, g0:g1],
                    in_=tanh_sb[:, g0:g1],
                    func=mybir.ActivationFunctionType.Exp,
                )

        # ---- epilogue: mask, normalize ----
        emm_sb = const.tile([P, CF], f32)
        rowsum = const.tile([P, 1], f32)
        nc.vector.scalar_tensor_tensor(
            out=emm_sb[:],
            in0=em_sb[:],
            scalar=0.0,
            in1=maskr_sb,
            op0=byp,
            op1=mul,
            accum_out=rowsum[:],
        )
        rowsum16 = const.tile([P, 1], f16)
        nc.vector.tensor_copy(rowsum16[:], rowsum[:])
        den_ps = psum.tile([P, 1], f32)
        nc.tensor.matmul(
            den_ps[:], lhsT=grp_sb[:], rhs=rowsum16[:], start=True, stop=True
        )
        rinv = const.tile([P, 1], f32)
        nc.vector.reciprocal(out=rinv[:], in_=den_ps[:])
        attn_sb = const.tile([P, CF], f32)
        nc.vector.tensor_scalar_mul(attn_sb[:], emm_sb[:], rinv[:])
        nc.sync.dma_start(out=out_e[:], in_=attn_sb[:])

    nc.compile()
    return nc


def _get_nc():
    if "nc" not in _STATE:
        _STATE["nc"] = _build_nc()
    return _STATE["nc"]


def _grp():
    if "GRP" not in _STATE:
        # GRP[p, m] = 1 iff p // J == m // J  (block-diagonal group-sum)
        pj = np.arange(P) // J
        _STATE["GRP"] = np.ascontiguousarray(
            (pj[:, None] == pj[None, :]).astype(np.float16)
        )
    return _STATE["GRP"]


def _make_in_maps(query, key, mask, W, bias):
    query = np.asarray(query, dtype=np.float32)
    key = np.asarray(key, dtype=np.float32)
    mask = np.asarray(mask, dtype=np.float32)
    W = np.asarray(W, dtype=np.float32)
    bias = np.asarray(bias, dtype=np.float32).reshape(-1)

    # wt[p, qc, k] = W.T[qc*128 + p, k]
    WT = np.ascontiguousarray(
        np.ascontiguousarray(W.T).reshape(QC, P, K).transpose(1, 0, 2)
    ).astype(np.float16)
    GRP = _grp()

    in_maps = []
    for i in range(NCORES):
        sh = slice(i * BS, (i + 1) * BS)
        maskbias = np.concatenate(
            [
                np.ascontiguousarray(mask[sh]).reshape(P, CF),
                np.broadcast_to(bias[:1][None, :], (P, 1)),
            ],
            axis=1,
        ).astype(np.float32)
        in_maps.append(
            {
                # pre-laid [P, QC, BS]: qt[p, qc, b] = query[sh].T[qc*128+p, b]
                "qt": np.ascontiguousarray(
                    query[sh].T.reshape(QC, P, BS).transpose(1, 0, 2)
                ).astype(np.float16),
                "wt": WT,
                "grp": GRP,
                "key": np.ascontiguousarray(key[sh]),
                "maskbias": np.ascontiguousarray(maskbias),
            }
        )
    return in_maps


def _run(in_maps, **kwargs):
    from concourse.bass_utils import run_bass_kernel_spmd

    return run_bass_kernel_spmd(
        _get_nc(), in_maps, core_ids=list(range(NCORES)), **kwargs
    )


def _gather(results):
    return np.concatenate(
        [np.asarray(r["out"]).reshape(BS, T) for r in results], axis=0
    )


def kernel(query, key, mask, W, bias):
    in_maps = _make_in_maps(query, key, mask, W, bias)
    res = _run(in_maps)
    return _gather(res.results)


# revision 44
# speedup vs baseline: 1.1958x; 1.0178x over previous
"""Trainium2 Bass kernel for masked-softmax attention scoring.

Reference computation (B=128, T=512, K=1024, Q=1024):
    mids  = einsum("kq,bq->bk", W, query)
    s     = tanh(einsum("btk,bk->bt", key, mids) + bias)
    attn  = softmax-like: exp(s - max) * mask / sum(exp(s - max) * mask)

The max-subtraction cancels exactly in the ratio (tanh is bounded), so the
device computes  attn = exp(tanh(.)) * mask / sum_t(exp(tanh(.)) * mask).

Sharding: data-parallel over B across 8 NeuronCores (16 batches/core).
Per-core layout: partition p = (b, j) with b in [0,16), j in [0,8);
free column c in [0,64); timestep t = j*64 + c.

Pipeline (single sync HWDGE ring, ~410 GB/s sustained):
 - ring order [qt, W^T x8, key col 0..63] guarantees W fully lands before
   any key bytes compete for bandwidth (the SDMA packet round-robin is not
   fair across rings, so a second ring is used only for the tiny
   mask/bias/grp loads).
 - W and query are fp16 (host-cast): halves W bytes and fp16 matmuls are
   single-pass (fp32 is a LOW+HIGH double pass).  PE warm-up matmuls on
   garbage data raise the PE pstate before the real mids chain.
 - mids matmul: stationary = 8x-replicated query columns; 16 accumulating
   matmuls paced by W chunk arrivals; PSUM -> SBUF copy on DVE.
 - each key column is consumed by ONE fused DVE multiply-reduce
   (native scalar_tensor_tensor, ~1.22 us/col = the stream rate; every
   DVE op with an accumulator runs 1x, and GpSimd/ACT cannot help:
   GpSimd tensor ops wreck DVE throughput via SBUF contention).
 - epilogue: tanh/exp per column-half on ACT (first half overlaps the
   stream), one fused mask-mul+rowsum (DVE), group-sum via block-diagonal
   0/1 fp16 matmul, reciprocal, scale, store.
"""

import sys

if "/opt/trn_rl_repo" not in sys.path:
    sys.path.insert(0, "/opt/trn_rl_repo")

from contextlib import ExitStack

import numpy as np

# ---- problem constants (hardcoded per spec) ----
B, T, K, Q = 128, 512, 1024, 1024
NCORES = 8
BS = B // NCORES          # 16 batches per core
P = 128                   # SBUF partitions
J = P // BS               # 8 t-blocks per batch on partitions
CF = T // J               # 64 timesteps per (partition, free col)
QC = Q // P               # 8 contraction chunks for the mids matmul
KEY_BUFS = 28             # key column pool depth (512 KB per slot)
N_WARM = 8                # PE pstate warm-up matmuls

_STATE: dict = {}


def _build_nc():
    import concourse.tile as tile
    from concourse import bacc, mybir

    f32 = mybir.dt.float32
    f16 = mybir.dt.float16
    mul = mybir.AluOpType.mult
    byp = mybir.AluOpType.bypass
    nc = bacc.Bacc()

    qt_e = nc.declare_dram_parameter("qt", [P, QC, BS], f16, isOutput=False)
    wt_e = nc.declare_dram_parameter("wt", [P, QC, K], f16, isOutput=False)
    grp_e = nc.declare_dram_parameter("grp", [P, P], f16, isOutput=False)
    key_e = nc.declare_dram_parameter("key", [BS, T, K], f32, isOutput=False)
    mb_e = nc.declare_dram_parameter("maskbias", [P, CF + 1], f32, isOutput=False)
    out_e = nc.declare_dram_parameter("out", [P, CF], f32, isOutput=True)

    with tile.TileContext(nc) as tc, ExitStack() as ctx:
        const = ctx.enter_context(tc.tile_pool(name="const", bufs=1))
        kpool = ctx.enter_context(tc.tile_pool(name="key", bufs=KEY_BUFS))
        vpool = ctx.enter_context(tc.tile_pool(name="vprod", bufs=2))
        psum = ctx.enter_context(tc.tile_pool(name="psum", bufs=1, space="PSUM"))

        # ---- PE pstate warm-up: garbage matmuls while the ring fills ----
        warm_sb = const.tile([P, 512], f16)
        nc.gpsimd.memset(warm_sb[:], 1.0)
        warm_ps = psum.tile([P, 512], f32)
        for _ in range(N_WARM):
            nc.tensor.matmul(
                warm_ps[:],
                lhsT=warm_sb[:, 0:P],
                rhs=warm_sb[:],
                start=True,
                stop=True,
            )

        # ---- prologue: everything big on the sync ring, tiny loads on the
        # scalar ring (cross-ring packet arbitration is unfair, so W must
        # not share a ring with key traffic).
        qt_sb = const.tile([P, QC, BS], f16)
        nc.sync.dma_start(out=qt_sb[:], in_=qt_e[:])
        mb_sb = const.tile([P, CF + 1], f32)
        nc.scalar.dma_start(out=mb_sb[:], in_=mb_e[:])
        grp_sb = const.tile([P, P], f16)
        nc.scalar.dma_start(out=grp_sb[:], in_=grp_e[:])
        maskr_sb = mb_sb[:, 0:CF]
        bias_sb = mb_sb[:, CF : CF + 1]
        wt_sb = const.tile([P, QC, K], f16)
        for qc in range(QC):
            nc.sync.dma_start(out=wt_sb[:, qc, :], in_=wt_e[:, qc, :])

        # ---- mids in broadcast layout: [P, K], row p = mids[b(p), :] ----
        # Replicate each query column 8x on-chip (stride-0 DVE read) so the
        # stationary operand has the (b, j) partition order in one free dim.
        qtrep_sb = const.tile([P, QC, BS, J], f16)
        nc.vector.tensor_copy(
            qtrep_sb[:], qt_sb[:].unsqueeze(-1).broadcast_to((P, QC, BS, J))
        )
        # two independent accumulation groups (k-halves); the h=0 copy runs
        # on the idle ACT engine in parallel with the DVE h=1 copy
        mids_ps0 = psum.tile([P, 512], f32)
        mids_ps1 = psum.tile([P, 512], f32)
        for qi, qc in enumerate(range(QC)):
            lhsT = qtrep_sb[:, qc, :, :]
            for h, ps in enumerate((mids_ps0, mids_ps1)):
                nc.tensor.matmul(
                    ps[:],
                    lhsT=lhsT,
                    rhs=wt_sb[:, qc, h * 512 : (h + 1) * 512],
                    start=(qi == 0),
                    stop=(qi == QC - 1),
                )
        mids_bc = const.tile([P, K], f32)
        nc.scalar.copy(out=mids_bc[:, 0:512], in_=mids_ps0[:])
        nc.vector.tensor_copy(mids_bc[:, 512:1024], mids_ps1[:])

        # ---- scores[p, c] = key[b, j*64+c, :] . mids[b, :] ----
        # 64 per-column 512 KB DMAs on the sync ring behind W; one fused
        # DVE multiply-reduce per column at the stream rate.
        scores_sb = const.tile([P, CF], f32)
        tanh_sb = const.tile([P, CF], f32)
        em_sb = const.tile([P, CF], f32)
        key_r = key_e[:].rearrange("b (j c) k -> (b j) c k", j=J)
        H = CF // 2
        for c in range(CF):
            kt = kpool.tile([P, K], f32, tag="k")
            nc.sync.dma_start(out=kt[:], in_=key_r[:, c, :])
            prod = vpool.tile([P, K], f16, tag="v")
            nc.vector.scalar_tensor_tensor(
                out=prod[:],
                in0=kt[:],
                scalar=0.0,
                in1=mids_bc[:],
                op0=byp,
                op1=mul,
                accum_out=scores_sb[:, c : c + 1],
            )
            if c == H - 1 or c == CF - 1:
                # tanh+exp per half on the (idle) ACT engine: the first
                # half overlaps the stream, only the second is tail work
                g0, g1 = (0, H) if c == H - 1 else (H, CF)
                nc.scalar.activation(
                    out=tanh_sb[:, g0:g1],
                    in_=scores_sb[:, g0:g1],
                    func=mybir.ActivationFunctionType.Tanh,
                    bias=bias_sb,
                    scale=1.0,
                )
                nc.scalar.activation(
                    out=em_sb[:, g0:g1],
                    in_=tanh_sb[:, g0:g1],
                    func=mybir.ActivationFunctionType.Exp,
                )

        # ---- epilogue: mask, normalize ----
        emm_sb = const.tile([P, CF], f32)
        rowsum = const.tile([P, 1], f32)
        nc.vector.scalar_tensor_tensor(
            out=emm_sb[:],
            in0=em_sb[:],
            scalar=0.0,
            in1=maskr_sb,
            op0=byp,
            op1=mul,
            accum_out=rowsum[:],
        )
        rowsum16 = const.tile([P, 1], f16)
        nc.vector.tensor_copy(rowsum16[:], rowsum[:])
        den_ps = psum.tile([P, 1], f32)
        nc.tensor.matmul(
            den_ps[:], lhsT=grp_sb[:], rhs=rowsum16[:], start=True, stop=True
        )
        rinv = const.tile([P, 1], f32)
        nc.vector.reciprocal(out=rinv[:], in_=den_ps[:])
        attn_sb = const.tile([P, CF], f32)
        nc.vector.tensor_scalar_mul(attn_sb[:], emm_sb[:], rinv[:])
        nc.sync.dma_start(out=out_e[:], in_=attn_sb[:])

    nc.compile()
    return nc


def _get_nc():
    if "nc" not in _STATE:
        _STATE["nc"] = _build_nc()
    return _STATE["nc"]


def _grp():
    if "GRP" not in _STATE:
        # GRP[p, m] = 1 iff p // J == m // J  (block-diagonal group-sum)
        pj = np.arange(P) // J
        _STATE["GRP"] = np.ascontiguousarray(
            (pj[:, None] == pj[None, :]).astype(np.float16)
        )
    return _STATE["GRP"]


def _make_in_maps(query, key, mask, W, bias):
    query = np.asarray(query, dtype=np.float32)
    key = np.asarray(key, dtype=np.float32)
    mask = np.asarray(mask, dtype=np.float32)
    W = np.asarray(W, dtype=np.float32)
    bias = np.asarray(bias, dtype=np.float32).reshape(-1)

    # wt[p, qc, k] = W.T[qc*128 + p, k]
    WT = np.ascontiguousarray(
        np.ascontiguousarray(W.T).reshape(QC, P, K).transpose(1, 0, 2)
    ).astype(np.float16)
    GRP = _grp()

    in_maps = []
    for i in range(NCORES):
        sh = slice(i * BS, (i + 1) * BS)
        maskbias = np.concatenate(
            [
                np.ascontiguousarray(mask[sh]).reshape(P, CF),
                np.broadcast_to(bias[:1][None, :], (P, 1)),
            ],
            axis=1,
        ).astype(np.float32)
        in_maps.append(
            {
                # pre-laid [P, QC, BS]: qt[p, qc, b] = query[sh].T[qc*128+p, b]
                "qt": np.ascontiguousarray(
                    query[sh].T.reshape(QC, P, BS).transpose(1, 0, 2)
                ).astype(np.float16),
                "wt": WT,
                "grp": GRP,
                "key": np.ascontiguousarray(key[sh]),
                "maskbias": np.ascontiguousarray(maskbias),
            }
        )
    return in_maps


def _run(in_maps, **kwargs):
    from concourse.bass_utils import run_bass_kernel_spmd

    return run_bass_kernel_spmd(
        _get_nc(), in_maps, core_ids=list(range(NCORES)), **kwargs
    )


def _gather(results):
    return np.concatenate(
        [np.asarray(r["out"]).reshape(BS, T) for r in results], axis=0
    )


def kernel(query, key, mask, W, bias):
    in_maps = _make_in_maps(query, key, mask, W, bias)
    res = _run(in_maps)
    return _gather(res.results)


# revision 45
# speedup vs baseline: 1.2051x; 1.0078x over previous
"""Trainium2 Bass kernel for masked-softmax attention scoring.

Reference computation (B=128, T=512, K=1024, Q=1024):
    mids  = einsum("kq,bq->bk", W, query)
    s     = tanh(einsum("btk,bk->bt", key, mids) + bias)
    attn  = softmax-like: exp(s - max) * mask / sum(exp(s - max) * mask)

The max-subtraction cancels exactly in the ratio (tanh is bounded), so the
device computes  attn = exp(tanh(.)) * mask / sum_t(exp(tanh(.)) * mask).

Sharding: data-parallel over B across 8 NeuronCores (16 batches/core).
Per-core layout: partition p = (b, j) with b in [0,16), j in [0,8);
free column c in [0,64); timestep t = j*64 + c.

Pipeline (single sync HWDGE ring, ~410 GB/s sustained):
 - ring order [qt, W^T x8, key col 0..63] guarantees W fully lands before
   any key bytes compete for bandwidth (the SDMA packet round-robin is not
   fair across rings, so a second ring is used only for the tiny
   mask/bias/grp loads).
 - W and query are fp16 (host-cast): halves W bytes and fp16 matmuls are
   single-pass (fp32 is a LOW+HIGH double pass).  PE warm-up matmuls on
   garbage data raise the PE pstate before the real mids chain.
 - mids matmul: stationary = 8x-replicated query columns; 16 accumulating
   matmuls paced by W chunk arrivals; PSUM -> SBUF copy on DVE.
 - each key column is consumed by ONE fused DVE multiply-reduce
   (native scalar_tensor_tensor, ~1.22 us/col = the stream rate; every
   DVE op with an accumulator runs 1x, and GpSimd/ACT cannot help:
   GpSimd tensor ops wreck DVE throughput via SBUF contention).
 - epilogue: tanh/exp per column-half on ACT (first half overlaps the
   stream), one fused mask-mul+rowsum (DVE), group-sum via block-diagonal
   0/1 fp16 matmul, reciprocal, scale, store.
"""

import sys

if "/opt/trn_rl_repo" not in sys.path:
    sys.path.insert(0, "/opt/trn_rl_repo")

from contextlib import ExitStack

import numpy as np

# ---- problem constants (hardcoded per spec) ----
B, T, K, Q = 128, 512, 1024, 1024
NCORES = 8
BS = B // NCORES          # 16 batches per core
P = 128                   # SBUF partitions
J = P // BS               # 8 t-blocks per batch on partitions
CF = T // J               # 64 timesteps per (partition, free col)
QC = Q // P               # 8 contraction chunks for the mids matmul
KEY_BUFS = 28             # key column pool depth (512 KB per slot)
N_WARM = 7                # PE pstate warm-up matmuls

_STATE: dict = {}


def _build_nc():
    import concourse.tile as tile
    from concourse import bacc, mybir

    f32 = mybir.dt.float32
    f16 = mybir.dt.float16
    mul = mybir.AluOpType.mult
    byp = mybir.AluOpType.bypass
    nc = bacc.Bacc()

    qt_e = nc.declare_dram_parameter("qt", [P, QC, BS], f16, isOutput=False)
    wt_e = nc.declare_dram_parameter("wt", [P, QC, K], f16, isOutput=False)
    grp_e = nc.declare_dram_parameter("grp", [P, P], f16, isOutput=False)
    key_e = nc.declare_dram_parameter("key", [BS, T, K], f32, isOutput=False)
    mb_e = nc.declare_dram_parameter("maskbias", [P, CF + 1], f32, isOutput=False)
    out_e = nc.declare_dram_parameter("out", [P, CF], f32, isOutput=True)

    with tile.TileContext(nc) as tc, ExitStack() as ctx:
        const = ctx.enter_context(tc.tile_pool(name="const", bufs=1))
        kpool = ctx.enter_context(tc.tile_pool(name="key", bufs=KEY_BUFS))
        vpool = ctx.enter_context(tc.tile_pool(name="vprod", bufs=2))
        psum = ctx.enter_context(tc.tile_pool(name="psum", bufs=1, space="PSUM"))

        # ---- PE pstate warm-up: garbage matmuls while the ring fills ----
        warm_sb = const.tile([P, 512], f16)
        nc.gpsimd.memset(warm_sb[:], 1.0)
        warm_ps = psum.tile([P, 512], f32)
        for _ in range(N_WARM):
            nc.tensor.matmul(
                warm_ps[:],
                lhsT=warm_sb[:, 0:P],
                rhs=warm_sb[:],
                start=True,
                stop=True,
            )

        # ---- prologue: everything big on the sync ring, tiny loads on the
        # scalar ring (cross-ring packet arbitration is unfair, so W must
        # not share a ring with key traffic).
        qt_sb = const.tile([P, QC, BS], f16)
        nc.sync.dma_start(out=qt_sb[:], in_=qt_e[:])
        mb_sb = const.tile([P, CF + 1], f32)
        nc.scalar.dma_start(out=mb_sb[:], in_=mb_e[:])
        grp_sb = const.tile([P, P], f16)
        nc.scalar.dma_start(out=grp_sb[:], in_=grp_e[:])
        maskr_sb = mb_sb[:, 0:CF]
        bias_sb = mb_sb[:, CF : CF + 1]
        wt_sb = const.tile([P, QC, K], f16)
        for qc in range(QC):
            nc.sync.dma_start(out=wt_sb[:, qc, :], in_=wt_e[:, qc, :])

        # ---- mids in broadcast layout: [P, K], row p = mids[b(p), :] ----
        # Replicate each query column 8x on-chip (stride-0 DVE read) so the
        # stationary operand has the (b, j) partition order in one free dim.
        qtrep_sb = const.tile([P, QC, BS, J], f16)
        nc.vector.tensor_copy(
            qtrep_sb[:], qt_sb[:].unsqueeze(-1).broadcast_to((P, QC, BS, J))
        )
        # two independent accumulation groups (k-halves); the h=0 copy runs
        # on the idle ACT engine in parallel with the DVE h=1 copy
        mids_ps0 = psum.tile([P, 512], f32)
        mids_ps1 = psum.tile([P, 512], f32)
        for qi, qc in enumerate(range(QC)):
            lhsT = qtrep_sb[:, qc, :, :]
            for h, ps in enumerate((mids_ps0, mids_ps1)):
                nc.tensor.matmul(
                    ps[:],
                    lhsT=lhsT,
                    rhs=wt_sb[:, qc, h * 512 : (h + 1) * 512],
                    start=(qi == 0),
                    stop=(qi == QC - 1),
                )
        mids_bc = const.tile([P, K], f32)
        nc.scalar.copy(out=mids_bc[:, 0:512], in_=mids_ps0[:])
        nc.vector.tensor_copy(mids_bc[:, 512:1024], mids_ps1[:])

        # ---- scores[p, c] = key[b, j*64+c, :] . mids[b, :] ----
        # 64 per-column 512 KB DMAs on the sync ring behind W; one fused
        # DVE multiply-reduce per column at the stream rate.
        scores_sb = const.tile([P, CF], f32)
        tanh_sb = const.tile([P, CF], f32)
        em_sb = const.tile([P, CF], f32)
        key_r = key_e[:].rearrange("b (j c) k -> (b j) c k", j=J)
        H = CF // 2
        for c in range(CF):
            kt = kpool.tile([P, K], f32, tag="k")
            nc.sync.dma_start(out=kt[:], in_=key_r[:, c, :])
            prod = vpool.tile([P, K], f16, tag="v")
            nc.vector.scalar_tensor_tensor(
                out=prod[:],
                in0=kt[:],
                scalar=0.0,
                in1=mids_bc[:],
                op0=byp,
                op1=mul,
                accum_out=scores_sb[:, c : c + 1],
            )
            if c == H - 1 or c == CF - 1:
                # tanh+exp per half on the (idle) ACT engine: the first
                # half overlaps the stream, only the second is tail work
                g0, g1 = (0, H) if c == H - 1 else (H, CF)
                nc.scalar.activation(
                    out=tanh_sb[:, g0:g1],
                    in_=scores_sb[:, g0:g1],
                    func=mybir.ActivationFunctionType.Tanh,
                    bias=bias_sb,
                    scale=1.0,
                )
                nc.scalar.activation(
                    out=em_sb[:, g0:g1],
                    in_=tanh_sb[:, g0:g1],
                    func=mybir.ActivationFunctionType.Exp,
                )

        # ---- epilogue: mask, normalize ----
        emm_sb = const.tile([P, CF], f32)
        rowsum = const.tile([P, 1], f32)
        nc.vector.scalar_tensor_tensor(
            out=emm_sb[:],
            in0=em_sb[:],
            scalar=0.0,
            in1=maskr_sb,
            op0=byp,
            op1=mul,
            accum_out=rowsum[:],
        )
        rowsum16 = const.tile([P, 1], f16)
        nc.vector.tensor_copy(rowsum16[:], rowsum[:])
        den_ps = psum.tile([P, 1], f32)
        nc.tensor.matmul(
            den_ps[:], lhsT=grp_sb[:], rhs=rowsum16[:], start=True, stop=True
        )
        rinv = const.tile([P, 1], f32)
        nc.vector.reciprocal(out=rinv[:], in_=den_ps[:])
        attn_sb = const.tile([P, CF], f32)
        nc.vector.tensor_scalar_mul(attn_sb[:], emm_sb[:], rinv[:])
        nc.sync.dma_start(out=out_e[:], in_=attn_sb[:])

    nc.compile()
    return nc


def _get_nc():
    if "nc" not in _STATE:
        _STATE["nc"] = _build_nc()
    return _STATE["nc"]


def _grp():
    if "GRP" not in _STATE:
        # GRP[p, m] = 1 iff p // J == m // J  (block-diagonal group-sum)
        pj = np.arange(P) // J
        _STATE["GRP"] = np.ascontiguousarray(
            (pj[:, None] == pj[None, :]).astype(np.float16)
        )
    return _STATE["GRP"]


def _make_in_maps(query, key, mask, W, bias):
    query = np.asarray(query, dtype=np.float32)
    key = np.asarray(key, dtype=np.float32)
    mask = np.asarray(mask, dtype=np.float32)
    W = np.asarray(W, dtype=np.float32)
    bias = np.asarray(bias, dtype=np.float32).reshape(-1)

    # wt[p, qc, k] = W.T[qc*128 + p, k]
    WT = np.ascontiguousarray(
        np.ascontiguousarray(W.T).reshape(QC, P, K).transpose(1, 0, 2)
    ).astype(np.float16)
    GRP = _grp()

    in_maps = []
    for i in range(NCORES):
        sh = slice(i * BS, (i + 1) * BS)
        maskbias = np.concatenate(
            [
                np.ascontiguousarray(mask[sh]).reshape(P, CF),
                np.broadcast_to(bias[:1][None, :], (P, 1)),
            ],
            axis=1,
        ).astype(np.float32)
        in_maps.append(
            {
                # pre-laid [P, QC, BS]: qt[p, qc, b] = query[sh].T[qc*128+p, b]
                "qt": np.ascontiguousarray(
                    query[sh].T.reshape(QC, P, BS).transpose(1, 0, 2)
                ).astype(np.float16),
                "wt": WT,
                "grp": GRP,
                "key": np.ascontiguousarray(key[sh]),
                "maskbias": np.ascontiguousarray(maskbias),
            }
        )
    return in_maps


def _run(in_maps, **kwargs):
    from concourse.bass_utils import run_bass_kernel_spmd

    return run_bass_kernel_spmd(
        _get_nc(), in_maps, core_ids=list(range(NCORES)), **kwargs
    )


def _gather(results):
    return np.concatenate(
        [np.asarray(r["out"]).reshape(BS, T) for r in results], axis=0
    )


def kernel(query, key, mask, W, bias):
    in_maps = _make_in_maps(query, key, mask, W, bias)
    res = _run(in_maps)
    return _gather(res.results)
